# revision 4
# baseline (speedup 1.0000x reference)
"""Trainium2 Bass kernel for nn_Attention_64974265254303.

Reference (T=S=H=O=1024, B=32):
    keys  = einsum('sbh,hl->sbl', hs, W_a)
    score = einsum('tbh,sbh->tbs', ht, keys)
    score = exp(score - max_s(score)); score[source.T==0] = 0
    a     = score / sum_s(score)
    c     = einsum('tbs,sbh->tbh', a, hs)
    out   = tanh(concat([c, ht], -1) @ W_c + b)

Strategy: pure data-parallel over batch (axis 1) -> 4 batches per core on 8
NeuronCores; W_a/W_c/b replicated. Per batch, all matmuls run in fp16 on the
TensorEngine (numerics verified: fp16 keys/score keeps final rel err ~3e-3
vs the 2e-2 budget; bf16 would be ~1.5e-2). Layouts are chosen so the only
transposes needed (ht, hs, and the attention matrix a) are done by the DMA
xbar (16x128-tile transpose of 2-byte data), keeping the PE free for matmuls.

Masked softmax: mask is applied additively (-30000 on masked columns) to the
score PSUM before exp, so exp underflows to exactly 0 there; exp runs on
ScalarE with per-row bias = -rowmax and a fused accumulated row-sum.
"""

import sys

for _p in ("/opt/trn_rl_repo",):
    if _p not in sys.path:
        sys.path.append(_p)

import numpy as np

import concourse.bass as bass
import concourse.tile as tile
from concourse import bacc, mybir
from concourse.bass_utils import run_bass_kernel_spmd

N_CORES = 8
T, S, B, H, O = 1024, 1024, 32, 1024, 1024
BL = B // N_CORES  # batches per core
PT = 128           # partition tile
NT = T // PT       # row tiles per matrix
NH = 512           # matmul free-dim half (one PSUM bank)
MASK_NEG = -30000.0

f32 = mybir.dt.float32
f16 = mybir.dt.float16
i32 = mybir.dt.int32


def _build(with_bias: bool):
    nc = bacc.Bacc("TRN2", target_bir_lowering=False, debug=False,
                   num_devices=N_CORES)

    ht_d = nc.dram_tensor("ht", [T, BL, H], f32, kind="ExternalInput").ap()
    hs_d = nc.dram_tensor("hs", [S, BL, H], f32, kind="ExternalInput").ap()
    src_d = nc.dram_tensor("src", [S, BL], i32, kind="ExternalInput").ap()
    wa_d = nc.dram_tensor("wa", [H, H], f32, kind="ExternalInput").ap()
    wc_d = nc.dram_tensor("wc", [2 * H, O], f32, kind="ExternalInput").ap()
    bias_d = (nc.dram_tensor("bias", [O], f32, kind="ExternalInput").ap()
              if with_bias else None)
    out_d = nc.dram_tensor("out", [T, BL, O], f32, kind="ExternalOutput").ap()

    with tile.TileContext(nc) as tc:
        with (
            tc.tile_pool(name="weights", bufs=1) as p_w,
            tc.tile_pool(name="in_f32", bufs=4) as p_in,
            tc.tile_pool(name="h16", bufs=3) as p_h16,
            tc.tile_pool(name="big16", bufs=1) as p_big,
            tc.tile_pool(name="mask", bufs=2) as p_mask,
            tc.tile_pool(name="ea", bufs=2) as p_e,
            tc.tile_pool(name="stats", bufs=8) as p_st,
            tc.tile_pool(name="outst", bufs=3) as p_out,
            tc.tile_pool(name="psA", bufs=4, space="PSUM") as p_psA,
            tc.tile_pool(name="psS", bufs=2, space="PSUM") as p_psS,
        ):
            # ---- weights: fp32 DRAM -> fp16 SBUF ----
            wa16 = p_w.tile([PT, NT, H], f16, tag="wa16")
            for kb in range(NT):
                w = p_in.tile([PT, H], f32, tag="inf32")
                nc.sync.dma_start(w[:], wa_d[bass.ts(kb, PT), :])
                nc.vector.tensor_copy(wa16[:, kb, :], w[:])
            wc16 = p_w.tile([PT, 2 * NT, O], f16, tag="wc16")
            for kb in range(2 * NT):
                w = p_in.tile([PT, O], f32, tag="inf32")
                nc.sync.dma_start(w[:], wc_d[bass.ts(kb, PT), :])
                nc.vector.tensor_copy(wc16[:, kb, :], w[:])

            bias_sb = None
            if with_bias:
                bias_sb = p_w.tile([1, O], f32, tag="biasrow")
                nc.sync.dma_start(bias_sb[:], bias_d.rearrange("(u o) -> u o", u=1))
                bias_bc = p_w.tile([PT, O], f32, tag="biasbc")
                nc.gpsimd.partition_broadcast(bias_bc[:], bias_sb[0:1, :])

            # ---- source -> per-batch additive mask rows [1, S] fp16 ----
            src_sb = p_w.tile([BL, S], i32, tag="src")
            nc.sync.dma_start(src_sb[:], src_d.rearrange("s b -> b s"))
            src_f = p_w.tile([BL, S], f32, tag="srcf")
            nc.vector.tensor_copy(src_f[:], src_sb[:])
            mrows = p_w.tile([BL, S], f16, tag="mrows")
            # (src == 0) * MASK_NEG, all batches at once (DVE must start at p0)
            nc.vector.tensor_scalar(
                mrows[:], src_f[:], 0.0, MASK_NEG,
                op0=mybir.AluOpType.is_equal, op1=mybir.AluOpType.mult,
            )

            for b in range(BL):
                # ---- load + cast + dma-xbar transpose ----
                # htT16[p, kb, t] = ht[t, 128*kb + p]; same for hsT16.
                htT16 = p_big.tile([PT, NT, T], f16, tag="htT")
                hsT16 = p_big.tile([PT, NT, S], f16, tag="hsT")
                hs16 = p_big.tile([PT, NT, H], f16, tag="hs16")  # [s, sb, h]
                for cb in range(NT):
                    htf = p_in.tile([PT, H], f32, tag="inf32")
                    nc.sync.dma_start(htf[:], ht_d[bass.ts(cb, PT), b, :])
                    ht16 = p_h16.tile([PT, H], f16, tag="ht16")
                    nc.vector.tensor_copy(ht16[:], htf[:])
                    nc.sync.dma_start(
                        htT16[:, :, bass.ts(cb, PT)], ht16[:], transpose=True)

                    hsf = p_in.tile([PT, H], f32, tag="inf32")
                    nc.sync.dma_start(hsf[:], hs_d[bass.ts(cb, PT), b, :])
                    nc.vector.tensor_copy(hs16[:, cb, :], hsf[:])
                    nc.sync.dma_start(
                        hsT16[:, :, bass.ts(cb, PT)], hs16[:, cb, :],
                        transpose=True)

                # broadcast this batch's mask row across partitions
                # (stage row b to partition 0 first; engines can't address
                # a start partition other than 0/32/64/96)
                mrow0 = p_mask.tile([1, S], f16, tag="mrow0")
                nc.sync.dma_start(mrow0[:], mrows[b : b + 1, :])
                maskb = p_mask.tile([PT, S], f16, tag="maskb")
                nc.gpsimd.partition_broadcast(maskb[:], mrow0[0:1, :])

                # ---- keys: keysT16[p, lb, s] = keys[s, 128*lb + p] ----
                keysT16 = p_big.tile([PT, NT, S], f16, tag="keysT")
                for lb in range(NT):
                    for sh in range(2):
                        ps = p_psA.tile([PT, NH], f32, tag="psA")
                        for kb in range(NT):
                            nc.tensor.matmul(
                                ps[:],
                                lhsT=wa16[:, kb, bass.ts(lb, PT)],
                                rhs=hsT16[:, kb, bass.ts(sh, NH)],
                                start=(kb == 0), stop=(kb == NT - 1),
                            )
                        nc.scalar.copy(keysT16[:, lb, bass.ts(sh, NH)], ps[:])

                # ---- score + masked softmax + aT ----
                # aT16[p, sb, t] = a[t, 128*sb + p]
                aT16 = p_big.tile([PT, NT, T], f16, tag="aT")
                for tb in range(NT):
                    sps = p_psS.tile([PT, S], f32, tag="psS")
                    for sh in range(2):
                        for lb in range(NT):
                            nc.tensor.matmul(
                                sps[:, bass.ts(sh, NH)],
                                lhsT=htT16[:, lb, bass.ts(tb, PT)],
                                rhs=keysT16[:, lb, bass.ts(sh, NH)],
                                start=(lb == 0), stop=(lb == NT - 1),
                            )
                    # additive mask, then rowmax/exp/rowsum
                    nc.vector.tensor_tensor(
                        sps[:], sps[:], maskb[:], op=mybir.AluOpType.add)
                    negmax = p_st.tile([PT, 1], f32, tag="negmax")
                    nc.vector.tensor_reduce(
                        negmax[:], sps[:], axis=mybir.AxisListType.X,
                        op=mybir.AluOpType.max, negate=True)
                    e16 = p_e.tile([PT, S], f16, tag="e16")
                    dsum = p_st.tile([PT, 1], f32, tag="dsum")
                    nc.scalar.activation(
                        e16[:], sps[:], mybir.ActivationFunctionType.Exp,
                        bias=negmax[:, 0:1], scale=1.0, accum_out=dsum[:, 0:1])
                    recip = p_st.tile([PT, 1], f32, tag="recip")
                    nc.vector.reciprocal(recip[:], dsum[:])
                    nc.vector.tensor_scalar_mul(e16[:], e16[:], recip[:, 0:1])
                    nc.sync.dma_start(
                        aT16[:, :, bass.ts(tb, PT)], e16[:], transpose=True)

                # ---- context: cT16[p, hb, t] = c[t, 128*hb + p] ----
                cT16 = p_big.tile([PT, NT, T], f16, tag="cT")
                for hb in range(NT):
                    for nh in range(2):
                        ps = p_psA.tile([PT, NH], f32, tag="psA")
                        for sb in range(NT):
                            nc.tensor.matmul(
                                ps[:],
                                lhsT=hs16[:, sb, bass.ts(hb, PT)],
                                rhs=aT16[:, sb, bass.ts(nh, NH)],
                                start=(sb == 0), stop=(sb == NT - 1),
                            )
                        nc.vector.tensor_copy(cT16[:, hb, bass.ts(nh, NH)], ps[:])

                # ---- z = concat(c, ht) @ W_c ; out = tanh(z + bias) ----
                for tb in range(NT):
                    for oh in range(2):
                        ps = p_psA.tile([PT, NH], f32, tag="psA")
                        for kb in range(2 * NT):
                            lhsT = (cT16[:, kb, bass.ts(tb, PT)] if kb < NT
                                    else htT16[:, kb - NT, bass.ts(tb, PT)])
                            nc.tensor.matmul(
                                ps[:], lhsT=lhsT,
                                rhs=wc16[:, kb, bass.ts(oh, NH)],
                                start=(kb == 0), stop=(kb == 2 * NT - 1),
                            )
                        if with_bias:
                            nc.vector.tensor_tensor(
                                ps[:], ps[:], bias_bc[:, bass.ts(oh, NH)],
                                op=mybir.AluOpType.add)
                        osb = p_out.tile([PT, NH], f32, tag="osb")
                        nc.scalar.activation(
                            osb[:], ps[:], mybir.ActivationFunctionType.Tanh)
                        nc.sync.dma_start(
                            out_d[bass.ts(tb, PT), b, bass.ts(oh, NH)], osb[:])

    nc.finalize()
    return nc


_NC_CACHE = {}


def _get_nc(with_bias: bool):
    if with_bias not in _NC_CACHE:
        _NC_CACHE[with_bias] = _build(with_bias)
    return _NC_CACHE[with_bias]


def _run(ht, hs, source, W_a, W_c, b, trace=False):
    ht = np.ascontiguousarray(np.asarray(ht, dtype=np.float32))
    hs = np.ascontiguousarray(np.asarray(hs, dtype=np.float32))
    source = np.asarray(source)
    W_a = np.ascontiguousarray(np.asarray(W_a, dtype=np.float32))
    W_c = np.ascontiguousarray(np.asarray(W_c, dtype=np.float32))
    b = np.ascontiguousarray(np.asarray(b, dtype=np.float32))
    src32 = np.ascontiguousarray(source.astype(np.int32))

    with_bias = bool(np.any(b))
    nc = _get_nc(with_bias)

    in_maps = []
    for i in range(N_CORES):
        sl = slice(i * BL, (i + 1) * BL)
        m = {
            "ht": np.ascontiguousarray(ht[:, sl, :]),
            "hs": np.ascontiguousarray(hs[:, sl, :]),
            "src": np.ascontiguousarray(src32[:, sl]),
            "wa": W_a,
            "wc": W_c,
        }
        if with_bias:
            m["bias"] = b
        in_maps.append(m)

    res = run_bass_kernel_spmd(
        nc, in_maps, core_ids=list(range(N_CORES)), trace=trace)
    out = np.concatenate([res.results[i]["out"] for i in range(N_CORES)],
                         axis=1)
    return out, res


def kernel(ht, hs, source, W_a, W_c, b):
    out, _ = _run(ht, hs, source, W_a, W_c, b, trace=False)
    return out
